# revision 2
# baseline (speedup 1.0000x reference)
"""HMM forward-algorithm log-likelihood kernel for Trainium2 (8 NeuronCores).

Problem: B=64 sequences, TMAX=2048 timesteps, N=256 hidden states, M=1024
emission symbols.  reference computes log p(x_b) via the log-domain forward
algorithm and gathers it at the last valid timestep T[b]-1.

Algorithm used here (mathematically equivalent, validated to ~1e-5 rel):
  *  Work in LINEAR space with the scaled forward recurrence
         v_{t} = Ehat[:, x_t] * (A @ v_{t-1})
     where A = softmax(trans, axis=0) (columns sum to 1) and
     Ehat = exp(log_softmax(emis,1) + lam) with a per-step scale e^lam chosen
     so log(sum v) stays near 0 (lam is calibrated at runtime on the host).
  *  Variable lengths: x is padded with an extra symbol (id M) whose emission
     column is exactly 1.0.  Since A is column-stochastic, padded steps
     preserve total mass exactly, so running all sequences a full 2048 steps
     leaves logsumexp(alpha_{T-1}) unchanged.  Host corrects by T[b]*lam.
  *  Time-chunked parallel scan: each sequence is split into KC=32 chunks of
     C=64 steps.  Chunks run in parallel as independent recurrence columns.
     Each chunk is preceded by BURN=16 burn-in steps starting from the ones
     vector; the forward direction contracts ~10x per step, so after 16 steps
     the direction error is below fp precision.  Per-chunk log-gains
     G_c = log(sum v_end) - log(sum v_start) telescope to the exact answer.
  *  Each of the 8 cores handles 8 sequences x 32 chunks = 256 columns.
     Per local step: 4 matmuls (256x256 A against 256 columns, split 2x2 into
     128-tiles) + a DVE elementwise multiply with the gathered emission
     columns (dma_gather transpose mode gathers Ehat rows into partition-dim
     columns directly).
Output of the device kernel: per-core (2, 256) fp32 of column sums at
s=BURN (Zs) and s=BURN+C (Ze).  Host combines gains, skips fully-padded
chunks, applies the lam correction, and returns (64, 1) float32.
"""

import numpy as np
import ml_dtypes

import concourse.bass as bass
import concourse.bacc as bacc
import concourse.tile as tile
import concourse.mybir as mybir
import concourse.bass_utils as bass_utils

BF16 = ml_dtypes.bfloat16

# Problem constants (hardcoded; kernel.py must be self-contained).
B, TMAX, N, M = 64, 2048, 256, 1024
NCORES = 8
BLOC = B // NCORES          # 8 sequences per core

# Algorithm parameters.
KC = 32                     # time-chunks per sequence
C = TMAX // KC              # 64 steps per chunk
BURN = 16                   # burn-in steps per chunk
STEPS = BURN + C            # 80 local steps
R = BLOC * KC               # 256 recurrence columns per core
NGRP = 2                    # ping-pong groups (overlap PE with DVE)
RG = R // NGRP              # 128 columns per group
W = 16                      # gather window (steps per dma_gather call)
NW = STEPS // W             # 5 windows

_CACHE = {}


def _log_softmax(a, axis):
    m = a.max(axis=axis, keepdims=True)
    s = a - m
    return s - np.log(np.exp(s).sum(axis=axis, keepdims=True))


def _build_program(act_evac):
    """Build the SPMD Bass program (same NEFF for all 8 cores)."""
    nc = bacc.Bacc(
        "TRN2",
        debug=False,
        enable_asserts=False,
        target_bir_lowering=False,
        num_devices=NCORES,
    )
    dt = mybir.dt

    at_d = nc.dram_tensor("at", [128, 2, 2, 128], dt.bfloat16, kind="ExternalInput")
    ehat_d = nc.dram_tensor("ehat", [M + 1, N], dt.bfloat16, kind="ExternalInput")
    pi_d = nc.dram_tensor("pi0", [128, 2, BLOC], dt.bfloat16, kind="ExternalInput")
    idx_d = nc.dram_tensor(
        "idx", [128, STEPS * R // 16], dt.int16, kind="ExternalInput"
    )
    zout_d = nc.dram_tensor("zout", [1, 2, R], dt.float32, kind="ExternalOutput")

    with tile.TileContext(nc) as tc:
        with (
            tc.tile_pool(name="singles", bufs=1) as singles,
            tc.tile_pool(name="state", bufs=1) as state,
            tc.tile_pool(name="eg", bufs=2) as egp,
            tc.tile_pool(name="work", bufs=2) as work,
            tc.tile_pool(name="ps", bufs=2, space="PSUM") as psp,
            tc.tile_pool(name="zps", bufs=2, space="PSUM") as zpsp,
        ):
            at_sb = singles.tile([128, 2, 2, 128], dt.bfloat16)
            nc.sync.dma_start(out=at_sb[:], in_=at_d.ap())
            pi_sb = singles.tile([128, 2, BLOC], dt.bfloat16)
            nc.sync.dma_start(out=pi_sb[:], in_=pi_d.ap())
            idx_sb = singles.tile([128, STEPS * R // 16], dt.int16)
            nc.sync.dma_start(out=idx_sb[:], in_=idx_d.ap())
            ones_sb = singles.tile([128, 1], dt.bfloat16)
            nc.vector.memset(ones_sb[:], 1.0)
            zbuf = singles.tile([1, 2, R], dt.float32)

            v = []
            for g in range(NGRP):
                vt = state.tile([128, 2, RG], dt.bfloat16, tag=f"v{g}")
                nc.vector.memset(vt[:], 1.0)
                v.append(vt)

            def gather(w):
                egt = egp.tile([128, 2, W * R], dt.bfloat16, tag="eg")
                nc.gpsimd.dma_gather(
                    egt[:],
                    ehat_d.ap(),
                    idx_sb[:, w * (W * R // 16):(w + 1) * (W * R // 16)],
                    W * R,
                    W * R,
                    N,
                    transpose=True,
                    single_packet=False,
                )
                return egt

            def snapshot(ev, grp, vt):
                zp = zpsp.tile([1, RG], dt.float32, tag="zps")
                nc.tensor.matmul(zp[:], ones_sb[:], vt[:, 0, :], start=True, stop=False)
                nc.tensor.matmul(zp[:], ones_sb[:], vt[:, 1, :], start=False, stop=True)
                nc.vector.tensor_copy(zbuf[:, ev, grp * RG:(grp + 1) * RG], zp[:])

            egt = gather(0)
            for w in range(NW):
                nxt = gather(w + 1) if w + 1 < NW else None
                for sl in range(W):
                    s = w * W + sl + 1
                    for g in range(NGRP):
                        vt = v[g]
                        ps = psp.tile([128, 2, RG], dt.float32, tag="ps")
                        for ic in range(2):
                            for kc in range(2):
                                nc.tensor.matmul(
                                    ps[:, ic, :],
                                    at_sb[:, kc, ic, :],
                                    vt[:, kc, :],
                                    start=(kc == 0),
                                    stop=(kc == 1),
                                )
                        egs = egt[:, :, sl * R + g * RG: sl * R + (g + 1) * RG]
                        if act_evac:
                            u = work.tile([128, 2, RG], dt.bfloat16, tag=f"u{g}")
                            nc.scalar.activation(
                                u[:], ps[:], mybir.ActivationFunctionType.Copy
                            )
                            nc.vector.tensor_mul(vt[:], u[:], egs)
                        else:
                            nc.vector.tensor_mul(vt[:], ps[:], egs)
                        if s == BURN and g == 0:
                            # chunk-0 columns are r = 0..BLOC-1 (group 0):
                            # overwrite with v_0 = Ehat[:, x[b,0]] * pi
                            nc.vector.tensor_mul(
                                vt[:, :, 0:BLOC],
                                egt[:, :, sl * R: sl * R + BLOC],
                                pi_sb[:],
                            )
                        if s == BURN:
                            snapshot(0, g, vt)
                        if s == STEPS:
                            snapshot(1, g, vt)
                egt = nxt
            nc.sync.dma_start(out=zout_d.ap(), in_=zbuf[:])

    nc.compile()
    return nc


def _prep_inputs(x, T, pi, trans, emis):
    """Host preprocessing: tables, lambda calibration, per-core index tensors."""
    x = np.asarray(x).astype(np.int64)
    T = np.asarray(T).astype(np.int64)
    pi = np.asarray(pi, dtype=np.float64)
    trans = np.asarray(trans, dtype=np.float64)
    emis = np.asarray(emis, dtype=np.float64)

    log_pi = _log_softmax(pi, 0)
    log_A = _log_softmax(trans, 0)
    log_E = _log_softmax(emis, 1)
    pi_exp = np.exp(log_pi)
    A_exp = np.exp(log_A)

    # lambda calibration: short fp32 run of the normalized recurrence.
    Af = A_exp.astype(np.float32)
    Ef = np.exp(log_E).astype(np.float32)
    nseq = min(16, B)
    v = np.ones((N, nseq), dtype=np.float32) / N
    acc = []
    ncal = min(48, int(T.max()))
    for t in range(1, max(2, ncal)):
        sym = x[:nseq, t]
        w_ = Ef[:, sym] * (Af @ v)
        Z = w_.sum(axis=0)
        Z = np.maximum(Z, 1e-30)
        acc.append(np.log(Z))
        v = w_ / Z
    tail = acc[len(acc) // 3:]
    lam = -float(np.mean(np.concatenate(tail))) if tail else 7.0

    # Tables.
    # at[k, kc, ic, i] = A_exp[ic*128 + i, kc*128 + k]   (lhsT tiles)
    at = np.empty((128, 2, 2, 128), dtype=BF16)
    for kc in range(2):
        for ic in range(2):
            blk = A_exp[ic * 128:(ic + 1) * 128, kc * 128:(kc + 1) * 128]
            at[:, kc, ic, :] = blk.T.astype(BF16)
    # ehat rows: [m, i];  row M is all-ones (pad symbol)
    ehat = np.ones((M + 1, N), dtype=BF16)
    ehat[:M, :] = np.exp(log_E + lam).T.astype(BF16)
    # pi tile: [p, c, b] = pi_exp[c*128 + p]
    pi_t = np.empty((128, 2, BLOC), dtype=BF16)
    for c in range(2):
        pi_t[:, c, :] = np.repeat(
            pi_exp[c * 128:(c + 1) * 128].astype(BF16)[:, None], BLOC, axis=1
        )

    # padded x: t in [0, 2048]; pad symbol M for t >= T[b]
    x_pad = np.full((B, TMAX + 1), M, dtype=np.int64)
    x_pad[:, :TMAX] = x
    for b in range(B):
        x_pad[b, T[b]:] = M

    # Per-core wrapped int16 index tensors.
    # column r = c*BLOC + b_loc ; global b = core*BLOC + b_loc
    # local step s (1..STEPS) applies transition t = c*C - BURN + s
    # t <= 0 -> pad ; except (c == 0, s == BURN) -> x[b, 0] (init overwrite)
    idx_tensors = []
    s_arr = np.arange(1, STEPS + 1)[:, None]          # (STEPS, 1)
    c_arr = (np.arange(R)[None, :] // BLOC)           # (1, R)
    b_arr = (np.arange(R)[None, :] % BLOC)            # (1, R)
    t_arr = c_arr * C - BURN + s_arr                  # (STEPS, R)
    for core in range(NCORES):
        bg = core * BLOC + b_arr                      # global b, (1, R)
        sym = np.where(
            (t_arr < 1) | (t_arr > TMAX),
            M,
            x_pad[np.broadcast_to(bg, t_arr.shape),
                  np.clip(t_arr, 1, TMAX)],
        )
        init_mask = (c_arr == 0) & (s_arr == BURN)
        sym = np.where(init_mask, x_pad[np.broadcast_to(bg, t_arr.shape), 0], sym)
        flat = sym.reshape(-1).astype(np.int16)       # j = (s-1)*R + r
        wrapped = flat.reshape(-1, 16).T              # (16, STEPS*R/16)
        idx = np.tile(wrapped, (8, 1))                # replicate for 8 Q7 cores
        idx_tensors.append(np.ascontiguousarray(idx))

    host = {
        "lam": lam,
        "T": T,
        "at": np.ascontiguousarray(at),
        "ehat": np.ascontiguousarray(ehat),
        "pi_t": np.ascontiguousarray(pi_t),
        "idx": idx_tensors,
    }
    return host


def _postprocess(zouts, lam, T):
    """Combine per-core (1, 2, R) Zs/Ze into (B, 1) float32 log-probs."""
    L = np.zeros(B, dtype=np.float64)
    for core in range(NCORES):
        z = np.asarray(zouts[core], dtype=np.float64).reshape(2, R)
        Zs, Ze = z[0], z[1]
        with np.errstate(divide="ignore", invalid="ignore"):
            G = np.log(Ze) - np.log(Zs)
        for b_loc in range(BLOC):
            b = core * BLOC + b_loc
            g = 0.0
            for c in range(KC):
                if c * C < T[b]:
                    g += G[c * BLOC + b_loc]
            L[b] = np.log(Zs[0 * BLOC + b_loc]) + g - T[b] * lam
    return L.reshape(B, 1).astype(np.float32)


def kernel(x, T, pi, trans, emis):
    host = _prep_inputs(x, T, pi, trans, emis)

    if "nc" not in _CACHE:
        _CACHE["nc"] = _build_program(act_evac=False)
    nc = _CACHE["nc"]

    in_maps = []
    for core in range(NCORES):
        in_maps.append(
            {
                "at": host["at"],
                "ehat": host["ehat"],
                "pi0": host["pi_t"],
                "idx": host["idx"][core],
            }
        )
    res = bass_utils.run_bass_kernel_spmd(nc, in_maps, core_ids=list(range(NCORES)))
    _CACHE["last_res"] = res
    zouts = [r["zout"] for r in res.results]
    return _postprocess(zouts, host["lam"], host["T"])



# revision 5
# speedup vs baseline: 7.5714x; 7.5714x over previous
"""HMM forward-algorithm log-likelihood kernel for Trainium2 (8 NeuronCores).

Problem: B=64 sequences, TMAX=2048 timesteps, N=256 hidden states, M=1024
emission symbols.  reference computes log p(x_b) via the log-domain forward
algorithm and gathers it at the last valid timestep T[b]-1.

Algorithm (mathematically equivalent, validated to ~1e-5 rel):
  *  Work in LINEAR space with the scaled forward recurrence
         v_{t} = Ehat[:, x_t] * (A @ v_{t-1})
     where A = softmax(trans, axis=0) (columns sum to 1) and
     Ehat = exp(log_softmax(emis,1) + lam) with a per-step scale e^lam chosen
     so log(sum v) stays near 0 (lam is calibrated at runtime on the host).
  *  Variable lengths: x is padded with an extra symbol (id M) whose emission
     column is exactly 1.0.  Since A is column-stochastic, padded steps
     preserve total mass exactly.  Host corrects by T[b]*lam.
  *  Time-chunked parallel scan: each sequence is split into chunks of C=16
     steps preceded by BURN=2 burn-in steps from the ones vector; the forward
     map contracts fast enough that the direction converges to below bf16
     precision.  Per-chunk log-gains G_c = log(sum v_end) - log(sum v_start)
     telescope to the exact answer.
  *  ONLY VALID CHUNKS ARE COMPUTED: chunk j of sequence b is scheduled only
     if j*C < T[b]; fully-padded chunks (whose gain the combine step would
     discard) are never assigned a column.  Columns are packed and
     load-balanced across the 8 cores on the host; which (b, j) a column
     represents is pure host-side labeling (emission stream + combine map).
  *  Each core runs R' columns as 4 independent chains so the PE stays
     saturated despite the serial matmul->multiply dependency.  Per
     chain-step: 4 matmuls (128x128 A tiles) into PSUM, then the emission
     multiply: chains 0-1 evacuate PSUM->SBUF bf16 on ScalarE followed by a
     2x-rate bf16 DVE multiply; chains 2-3 multiply straight out of PSUM on
     DVE (1x).  This splits the per-step elementwise load across both
     PSUM-capable engines.
  *  Emission columns are pre-gathered ON THE HOST into a per-core stream
     and DMA'd contiguously in 2-step windows, buffered 4 deep -- no
     device-side gather at all.
Output of the device kernel: per-core (1, 2, R') fp32 of column sums at
s=BURN (Zs) and s=STEPS (Ze).  Host combines gains per sequence, applies the
lam correction, and returns (64, 1) float32.
"""

import numpy as np
import ml_dtypes

import concourse.bass as bass
import concourse.bacc as bacc
import concourse.tile as tile
import concourse.mybir as mybir
import concourse.bass_utils as bass_utils

BF16 = ml_dtypes.bfloat16

# Problem constants (hardcoded; kernel.py must be self-contained).
B, TMAX, N, M = 64, 2048, 256, 1024
NCORES = 8
BLOC = B // NCORES          # 8 sequences per core

# Algorithm parameters.
C = 16                      # steps per chunk
BURN = 2                    # burn-in steps per chunk
STEPS = BURN + C            # 18 local steps
NCH = 4                     # independent chains
W = 2                       # steps per DMA window
NW = STEPS // W             # 9 windows
PREFETCH = 4                # window buffers in flight

_CACHE = {}


def _log_softmax(a, axis):
    m = a.max(axis=axis, keepdims=True)
    s = a - m
    return s - np.log(np.exp(s).sum(axis=axis, keepdims=True))


def _build_program(Rp):
    """Build the SPMD Bass program for R'=Rp columns per core."""
    CW = Rp // NCH
    nc = bacc.Bacc(
        "TRN2",
        debug=False,
        enable_asserts=False,
        target_bir_lowering=False,
        num_devices=NCORES,
    )
    dt = mybir.dt

    at_d = nc.dram_tensor("at", [128, 2, 2, 128], dt.bfloat16, kind="ExternalInput")
    pi_d = nc.dram_tensor("pi0", [128, 2, BLOC], dt.bfloat16, kind="ExternalInput")
    eg_d = nc.dram_tensor("eg", [NW, 128, 2, W * Rp], dt.bfloat16, kind="ExternalInput")
    zout_d = nc.dram_tensor("zout", [1, 2, Rp], dt.float32, kind="ExternalOutput")

    with tile.TileContext(nc) as tc:
        with (
            tc.tile_pool(name="singles", bufs=1) as singles,
            tc.tile_pool(name="state", bufs=1) as state,
            tc.tile_pool(name="work", bufs=1) as work,
            tc.tile_pool(name="eg", bufs=PREFETCH) as egp,
            tc.tile_pool(name="ps", bufs=1, space="PSUM") as psp,
            tc.tile_pool(name="zps", bufs=2, space="PSUM") as zpsp,
        ):
            at_sb = singles.tile([128, 2, 2, 128], dt.bfloat16)
            nc.sync.dma_start(out=at_sb[:], in_=at_d.ap())
            pi_sb = singles.tile([128, 2, BLOC], dt.bfloat16)
            nc.sync.dma_start(out=pi_sb[:], in_=pi_d.ap())
            ones_sb = singles.tile([128, 1], dt.bfloat16)
            nc.vector.memset(ones_sb[:], 1.0)
            zbuf = singles.tile([1, 2, Rp], dt.float32)

            v = []
            u = []
            ps = []
            for g in range(NCH):
                vt = state.tile([128, 2, CW], dt.bfloat16, tag=f"v{g}", name=f"v{g}")
                nc.vector.memset(vt[:], 1.0)
                v.append(vt)
                u.append(work.tile([128, 2, CW], dt.bfloat16, tag=f"u{g}", name=f"u{g}"))
                ps.append(psp.tile([128, 2, CW], dt.float32, tag=f"ps{g}", name=f"ps{g}"))

            def issue_eg(w):
                t = egp.tile([128, 2, W * Rp], dt.bfloat16, tag="eg")
                nc.sync.dma_start(out=t[:], in_=eg_d.ap()[w])
                return t

            def snapshot(ev, g, vt):
                zp = zpsp.tile([1, CW], dt.float32, tag="zps")
                nc.tensor.matmul(zp[:], ones_sb[:], vt[:, 0, :], start=True, stop=False)
                nc.tensor.matmul(zp[:], ones_sb[:], vt[:, 1, :], start=False, stop=True)
                nc.scalar.copy(zbuf[:, ev, g * CW:(g + 1) * CW], zp[:])

            egts = [None] * NW
            for w in range(min(PREFETCH, NW)):
                egts[w] = issue_eg(w)

            for w in range(NW):
                egt = egts[w]
                for sl in range(W):
                    s = w * W + sl + 1
                    for g in range(NCH):
                        pg = ps[g]
                        for ic in range(2):
                            for kc in range(2):
                                nc.tensor.matmul(
                                    pg[:, ic, :],
                                    at_sb[:, kc, ic, :],
                                    v[g][:, kc, :],
                                    start=(kc == 0),
                                    stop=(kc == 1),
                                )
                        base = sl * Rp + g * CW
                        egs = egt[:, :, base:base + CW]
                        if g < 2:
                            # ScalarE evacuation + 2x bf16 DVE multiply
                            nc.scalar.copy(u[g][:], pg[:])
                            nc.vector.tensor_mul(v[g][:], u[g][:], egs)
                        else:
                            # direct 1x PSUM multiply on DVE
                            nc.vector.tensor_mul(v[g][:], pg[:], egs)
                        if s == BURN and g == 0:
                            # chunk-0 columns are pinned at r = 0..BLOC-1
                            # (chain 0): overwrite with v_0 = Ehat[:, x0] * pi
                            nc.vector.tensor_mul(
                                v[0][:, :, 0:BLOC],
                                egt[:, :, sl * Rp: sl * Rp + BLOC],
                                pi_sb[:],
                            )
                        if s == BURN:
                            snapshot(0, g, v[g])
                        if s == STEPS:
                            snapshot(1, g, v[g])
                if w + PREFETCH < NW:
                    egts[w + PREFETCH] = issue_eg(w + PREFETCH)
            nc.sync.dma_start(out=zout_d.ap(), in_=zbuf[:])

    nc.compile()
    return nc


def _prep_inputs(x, T, pi, trans, emis):
    """Host preprocessing: tables, lambda calibration, packed column
    assignment, and per-core emission streams."""
    x = np.asarray(x).astype(np.int64)
    T = np.asarray(T).astype(np.int64)
    pi = np.asarray(pi, dtype=np.float64)
    trans = np.asarray(trans, dtype=np.float64)
    emis = np.asarray(emis, dtype=np.float64)

    log_pi = _log_softmax(pi, 0)
    log_A = _log_softmax(trans, 0)
    log_E = _log_softmax(emis, 1)
    pi_exp = np.exp(log_pi)
    A_exp = np.exp(log_A)

    # lambda calibration: short fp32 run of the normalized recurrence.
    Af = A_exp.astype(np.float32)
    Ef = np.exp(log_E).astype(np.float32)
    nseq = min(16, B)
    vcal = np.ones((N, nseq), dtype=np.float32) / N
    acc = []
    ncal = min(48, int(T.max()))
    for t in range(1, max(2, ncal)):
        sym = x[:nseq, t]
        w_ = Ef[:, sym] * (Af @ vcal)
        Z = np.maximum(w_.sum(axis=0), 1e-30)
        acc.append(np.log(Z))
        vcal = w_ / Z
    tail = acc[len(acc) // 3:]
    lam = -float(np.mean(np.concatenate(tail))) if tail else 7.0

    # Tables.
    # at[k, kc, ic, i] = A_exp[ic*128 + i, kc*128 + k]   (lhsT tiles)
    at = np.empty((128, 2, 2, 128), dtype=BF16)
    for kc in range(2):
        for ic in range(2):
            blk = A_exp[ic * 128:(ic + 1) * 128, kc * 128:(kc + 1) * 128]
            at[:, kc, ic, :] = blk.T.astype(BF16)
    # ehat rows: [m, i];  row M is all-ones (pad symbol)
    ehat = np.ones((M + 1, N), dtype=BF16)
    ehat[:M, :] = np.exp(log_E + lam).T.astype(BF16)
    # pi tile: [p, c, b] = pi_exp[c*128 + p]
    pi_t = np.empty((128, 2, BLOC), dtype=BF16)
    for c in range(2):
        pi_t[:, c, :] = np.repeat(
            pi_exp[c * 128:(c + 1) * 128].astype(BF16)[:, None], BLOC, axis=1
        )

    # padded x: t in [0, 2048]; pad symbol M for t >= T[b]
    x_pad = np.full((B, TMAX + 1), M, dtype=np.int64)
    x_pad[:, :TMAX] = x
    for b in range(B):
        x_pad[b, T[b]:] = M

    # ---- packed column assignment --------------------------------------
    # chunk j of sequence b handles transitions t in [j*C+1, (j+1)*C];
    # valid iff j*C < T[b] (the combine step discards the rest).
    nv = ((T + C - 1) // C).astype(np.int64)      # ceil(T/C) >= 1
    cols = [[] for _ in range(NCORES)]            # per-core list of (b, j)
    for b in range(B):
        cols[b // BLOC].append((b, 0))            # chunk 0 pinned at slot b%BLOC
    rest = [(b, j) for b in range(B) for j in range(1, int(nv[b]))]
    for i, bc in enumerate(rest):
        cols[i % NCORES].append(bc)
    need = max(len(c) for c in cols)
    Rp = NCH * int(np.ceil(need / NCH))
    if Rp % 2:
        Rp += NCH

    # Per-core symbol tables and emission streams.
    s_arr = np.arange(1, STEPS + 1)[:, None]      # (STEPS, 1)
    eg_tensors = []
    colmaps = []
    for core in range(NCORES):
        cl = cols[core]
        b_arr = np.full(Rp, 0, dtype=np.int64)
        j_arr = np.full(Rp, 0, dtype=np.int64)
        pad_col = np.ones(Rp, dtype=bool)
        for r, (b, j) in enumerate(cl):
            b_arr[r] = b
            j_arr[r] = j
            pad_col[r] = False
        t_arr = j_arr[None, :] * C - BURN + s_arr  # (STEPS, Rp)
        sym = np.where(
            pad_col[None, :] | (t_arr < 1) | (t_arr > TMAX),
            M,
            x_pad[np.broadcast_to(b_arr[None, :], t_arr.shape),
                  np.clip(t_arr, 1, TMAX)],
        )
        init_mask = (~pad_col[None, :]) & (j_arr[None, :] == 0) & (s_arr == BURN)
        sym = np.where(
            init_mask, x_pad[np.broadcast_to(b_arr[None, :], t_arr.shape), 0], sym
        )
        # gathered[(s-1), r, :] = ehat[sym] ; device layout eg[w, p, c2, sl*Rp+r]
        gath = ehat[sym.reshape(-1)]              # (STEPS*Rp, N) bf16
        gath = gath.reshape(NW, W, Rp, 2, 128)
        eg = np.ascontiguousarray(gath.transpose(0, 4, 3, 1, 2).reshape(
            NW, 128, 2, W * Rp))
        eg_tensors.append(eg)
        colmaps.append(cl)

    host = {
        "lam": lam,
        "T": T,
        "Rp": Rp,
        "at": np.ascontiguousarray(at),
        "pi_t": np.ascontiguousarray(pi_t),
        "eg": eg_tensors,
        "colmaps": colmaps,
    }
    return host


def _postprocess(zouts, lam, T, Rp, colmaps):
    """Combine per-core (1, 2, R') Zs/Ze into (B, 1) float32 log-probs."""
    logZs_0 = np.zeros(B, dtype=np.float64)       # chunk-0 log Zs per seq
    gain = np.zeros(B, dtype=np.float64)          # sum of valid chunk gains
    for core in range(NCORES):
        z = np.asarray(zouts[core], dtype=np.float64).reshape(2, Rp)
        Zs, Ze = z[0], z[1]
        with np.errstate(divide="ignore", invalid="ignore"):
            G = np.log(Ze) - np.log(Zs)
        for r, (b, j) in enumerate(colmaps[core]):
            gain[b] += G[r]
            if j == 0:
                logZs_0[b] = np.log(Zs[r])
    L = logZs_0 + gain - T * lam
    return L.reshape(B, 1).astype(np.float32)


def kernel(x, T, pi, trans, emis):
    host = _prep_inputs(x, T, pi, trans, emis)
    Rp = host["Rp"]

    key = ("nc", Rp)
    if key not in _CACHE:
        _CACHE[key] = _build_program(Rp)
    nc = _CACHE[key]

    in_maps = []
    for core in range(NCORES):
        in_maps.append(
            {
                "at": host["at"],
                "pi0": host["pi_t"],
                "eg": host["eg"][core],
            }
        )
    res = bass_utils.run_bass_kernel_spmd(nc, in_maps, core_ids=list(range(NCORES)))
    _CACHE["last_res"] = res
    zouts = [r["zout"] for r in res.results]
    return _postprocess(zouts, host["lam"], host["T"], Rp, host["colmaps"])
